# revision 1
# baseline (speedup 1.0000x reference)
"""GCNII forward on 8 TRN2 NeuronCores (self-contained).

Strategy (1D row partitioning per sharding hint):
- nodes sharded 2500/core (padded 2560); edges assigned to the core owning dst.
- per layer: ht = dinv*h exchanged as fp16 via one 8-rank AllGather into a
  pair-SHARED DRAM table [20480,1024]; each core indirect-DMA-gathers its
  edges' source rows (128 rows/instr), scatter-adds them into per-dst-tile
  PSUM via one-hot fp16 matmuls, then computes z = 0.9*dinv*agg + 0.1*h0 and
  the layer GEMM z @ ((1-b)I + b*W) in fp32r (TF32-like) with the identity
  residual folded into the weights on the host.
- self-loops are real edges; gcn_norm folded into per-node dinv scaling.
"""
import math
import numpy as np

import concourse.bass as bass
import concourse.mybir as mybir
import concourse.tile as tile
from concourse import bacc
from concourse.bass_utils import run_bass_kernel_spmd
from concourse.masks import make_identity

# problem constants (hardcoded per contract)
N, E = 20000, 320000
F_IN, H, C, L = 512, 1024, 64, 8
ALPHA, THETA = 0.1, 0.5
NCORES = 8
SH = N // NCORES          # 2500 real rows per core
SHP = 2560                # padded rows per core (20*128)
V = NCORES * SHP          # padded table rows
P = 128
NT = SHP // P             # 20 dst tiles per core
KF = F_IN // P            # 4 k-tiles for W1
KH = H // P               # 8 k-tiles for H

f32 = mybir.dt.float32
f32r = mybir.dt.float32r
f16 = mybir.dt.float16
i32 = mybir.dt.int32

_cache = {}


def _preprocess(x, edge_index, W1, b1, Wg, W2, b2):
    src = np.asarray(edge_index[0], dtype=np.int64)
    dst = np.asarray(edge_index[1], dtype=np.int64)
    # self loops
    loops = np.arange(N, dtype=np.int64)
    src = np.concatenate([src, loops])
    dst = np.concatenate([dst, loops])
    deg = np.bincount(dst, minlength=N).astype(np.float32)
    dinv = 1.0 / np.sqrt(np.maximum(deg, 1.0))

    core = dst // SH
    d_loc = dst - core * SH
    tl = d_loc // P
    slot = d_loc % P
    gid = core * NT + tl
    order = np.argsort(gid, kind="stable")
    gid_s = gid[order]
    src_s = src[order]
    slot_s = slot[order]
    # rank within group
    counts = np.bincount(gid_s, minlength=NCORES * NT)
    starts = np.concatenate([[0], np.cumsum(counts)[:-1]])
    j = np.arange(len(gid_s)) - starts[gid_s]
    nchunk = int(math.ceil(counts.max() / P))
    c_idx = j // P
    p_idx = j % P
    s_tab = ((src_s // SH) * SHP + (src_s % SH)).astype(np.int32)

    offs = np.zeros((NCORES, P, NT * nchunk), dtype=np.int32)
    S = np.zeros((NCORES, NT, P, nchunk, P), dtype=np.float16)  # [core,t,e,c,d]
    core_s = gid_s // NT
    tl_s = gid_s % NT
    offs[core_s, p_idx, tl_s * nchunk + c_idx] = s_tab
    S[core_s, tl_s, p_idx, c_idx, slot_s] = np.float16(1.0)
    S = S.reshape(NCORES, NT, P, nchunk * P)

    # per-core dinv columns [P, NT]
    dinv_pad = np.zeros(NCORES * SHP, dtype=np.float32)
    idx = np.arange(N)
    dinv_pad[(idx // SH) * SHP + (idx % SH)] = dinv
    dinvc = dinv_pad.reshape(NCORES, NT, P).transpose(0, 2, 1).copy()  # [c,P,NT]
    dinv09c = (0.9 * dinvc).astype(np.float32)

    # xT shards [F_IN, SHP] padded
    x = np.asarray(x, dtype=np.float32)
    xT = np.zeros((NCORES, F_IN, SHP), dtype=np.float32)
    for c in range(NCORES):
        xT[c, :, :SH] = x[c * SH:(c + 1) * SH].T

    betas = np.log(THETA / np.arange(1.0, L + 1.0, dtype=np.float64) + 1.0)
    Wg = np.asarray(Wg, dtype=np.float64)
    eye = np.eye(H, dtype=np.float64)
    Wt = np.stack([(1.0 - betas[l]) * eye + betas[l] * Wg[l] for l in range(L)])
    Wt = Wt.astype(np.float32)

    b1b = np.broadcast_to(np.asarray(b1, np.float32), (P, H)).copy()
    b2b = np.broadcast_to(np.asarray(b2, np.float32), (P, C)).copy()

    in_maps = []
    for c in range(NCORES):
        in_maps.append({
            "xT": xT[c],
            "W1": np.asarray(W1, np.float32),
            "Wt": Wt,
            "W2": np.asarray(W2, np.float32),
            "b1b": b1b,
            "b2b": b2b,
            "dinvc": dinvc[c],
            "dinv09c": dinv09c[c],
            "offs": offs[c],
            "Smat": S[c],
        })
    return in_maps, nchunk


def _build(nchunk):
    nc = bacc.Bacc("TRN2", target_bir_lowering=False, debug=False,
                   num_devices=NCORES)
    t_xT = nc.dram_tensor("xT", [F_IN, SHP], f32r, kind="ExternalInput")
    t_W1 = nc.dram_tensor("W1", [F_IN, H], f32r, kind="ExternalInput")
    t_Wt = nc.dram_tensor("Wt", [L, H, H], f32r, kind="ExternalInput")
    t_W2 = nc.dram_tensor("W2", [H, C], f32r, kind="ExternalInput")
    t_b1 = nc.dram_tensor("b1b", [P, H], f32, kind="ExternalInput")
    t_b2 = nc.dram_tensor("b2b", [P, C], f32, kind="ExternalInput")
    t_dinv = nc.dram_tensor("dinvc", [P, NT], f32, kind="ExternalInput")
    t_dinv09 = nc.dram_tensor("dinv09c", [P, NT], f32, kind="ExternalInput")
    t_offs = nc.dram_tensor("offs", [P, NT * nchunk], i32, kind="ExternalInput")
    t_S = nc.dram_tensor("Smat", [NT, P, nchunk * P], f16, kind="ExternalInput")
    t_out = nc.dram_tensor("out", [SHP, C], f32, kind="ExternalOutput")

    h0s_dram = nc.dram_tensor("h0s", [SHP, H], f32)
    exch_in = nc.dram_tensor("exch", [SHP, H], f16)
    tables = [nc.dram_tensor(f"tbl{i}", [V, H], f16, addr_space="Shared")
              for i in range(2)]

    with tile.TileContext(nc) as tc:
        with (
            tc.tile_pool(name="const", bufs=1) as cp,
            tc.tile_pool(name="wpool", bufs=1) as wp,
            tc.tile_pool(name="spool", bufs=2) as sp,
            tc.tile_pool(name="gpool", bufs=6) as gp,
            tc.tile_pool(name="zpool", bufs=2) as zp,
            tc.tile_pool(name="ps_agg", bufs=2, space="PSUM") as pa,
            tc.tile_pool(name="ps_gemm", bufs=1, space="PSUM") as pg,
            tc.tile_pool(name="ps_tr", bufs=2, space="PSUM") as pt,
        ):
            ident = cp.tile([P, P], f32, tag="ident")
            make_identity(nc, ident[:])
            offs_sb = cp.tile([P, NT * nchunk], i32, tag="offs")
            nc.sync.dma_start(out=offs_sb[:], in_=t_offs[:])
            dinv_sb = cp.tile([P, NT], f32, tag="dinv")
            nc.sync.dma_start(out=dinv_sb[:], in_=t_dinv[:])
            dinv09_sb = cp.tile([P, NT], f32, tag="dinv09")
            nc.sync.dma_start(out=dinv09_sb[:], in_=t_dinv09[:])
            b1_sb = cp.tile([P, H], f32, tag="b1")
            nc.sync.dma_start(out=b1_sb[:], in_=t_b1[:])
            b2_sb = cp.tile([P, C], f32, tag="b2")
            nc.sync.dma_start(out=b2_sb[:], in_=t_b2[:])

            # ---- phase 0: h0 = relu(x@W1 + b1); h0s = 0.1*h0; table0 = f16(dinv*h0)
            xT_sb = cp.tile([P, KF * SHP], f32r, tag="xT")
            for k in range(KF):
                nc.sync.dma_start(out=xT_sb[:, k * SHP:(k + 1) * SHP],
                                  in_=t_xT[k * P:(k + 1) * P, :])
            W_sb = wp.tile([P, KF * H], f32r, tag="W")
            for k in range(KF):
                nc.sync.dma_start(out=W_sb[:, k * H:(k + 1) * H],
                                  in_=t_W1[k * P:(k + 1) * P, :])
            for t in range(NT):
                ps = pg.tile([P, H], f32, space="PSUM", tag="gemm")
                for k in range(KF):
                    for nh in range(2):
                        nc.tensor.matmul(
                            out=ps[:, nh * 512:(nh + 1) * 512],
                            lhsT=xT_sb[:, k * SHP + t * P: k * SHP + (t + 1) * P],
                            rhs=W_sb[:, k * H + nh * 512: k * H + (nh + 1) * 512],
                            start=(k == 0), stop=(k == KF - 1))
                nc.vector.tensor_add(out=ps[:], in0=ps[:], in1=b1_sb[:])
                h0s_t = zp.tile([P, H], f32, tag="h0w")
                nc.scalar.activation(out=h0s_t[:], in_=ps[:],
                                     func=mybir.ActivationFunctionType.Relu,
                                     scale=0.1)
                nc.sync.dma_start(out=h0s_dram[t * P:(t + 1) * P, :], in_=h0s_t[:])
                ex_t = zp.tile([P, H], f16, tag="ex")
                nc.scalar.activation(out=ex_t[:], in_=ps[:],
                                     func=mybir.ActivationFunctionType.Relu,
                                     scale=dinv_sb[:, t:t + 1])
                nc.sync.dma_start(out=exch_in[t * P:(t + 1) * P, :], in_=ex_t[:])
            nc.gpsimd.collective_compute(
                "AllGather", mybir.AluOpType.bypass,
                replica_groups=[list(range(NCORES))],
                ins=[exch_in.ap().opt()], outs=[tables[0].ap().opt()])

            W2_sb = cp.tile([P, KH * C], f32r, tag="W2")
            for k in range(KH):
                nc.sync.dma_start(out=W2_sb[:, k * C:(k + 1) * C],
                                  in_=t_W2[k * P:(k + 1) * P, :])

            # ---- layers
            for l in range(L):
                tbl = tables[l % 2]
                W_sb = wp.tile([P, KH * H], f32r, tag="W")
                for k in range(KH):
                    nc.sync.dma_start(out=W_sb[:, k * H:(k + 1) * H],
                                      in_=t_Wt[l, k * P:(k + 1) * P, :])
                for t in range(NT):
                    S_sb = sp.tile([P, nchunk * P], f16, tag="S")
                    nc.sync.dma_start(out=S_sb[:], in_=t_S[t])
                    agg = pa.tile([P, H], f32, space="PSUM", tag="agg")
                    for c in range(nchunk):
                        g_sb = gp.tile([P, H], f16, tag="g")
                        nc.gpsimd.indirect_dma_start(
                            out=g_sb[:], out_offset=None, in_=tbl.ap(),
                            in_offset=bass.IndirectOffsetOnAxis(
                                ap=offs_sb[:, t * nchunk + c: t * nchunk + c + 1],
                                axis=0))
                        for nh in range(2):
                            nc.tensor.matmul(
                                out=agg[:, nh * 512:(nh + 1) * 512],
                                lhsT=S_sb[:, c * P:(c + 1) * P],
                                rhs=g_sb[:, nh * 512:(nh + 1) * 512],
                                start=(c == 0), stop=(c == nchunk - 1))
                    # z = 0.9*dinv*agg + 0.1*h0   (as f32r for the GEMM)
                    h0s_t = zp.tile([P, H], f32, tag="h0r")
                    nc.sync.dma_start(out=h0s_t[:],
                                      in_=h0s_dram[t * P:(t + 1) * P, :])
                    z0 = zp.tile([P, H], f32, tag="z0")
                    nc.vector.tensor_scalar(
                        out=z0[:], in0=agg[:], scalar1=dinv09_sb[:, t:t + 1],
                        scalar2=None, op0=mybir.AluOpType.mult)
                    z = zp.tile([P, H], f32, tag="z")
                    nc.vector.tensor_add(out=z[:], in0=z0[:], in1=h0s_t[:])
                    # transpose z -> zT (8 k-tiles)
                    zT = zp.tile([P, KH * P], f32r, tag="zT")
                    for k in range(KH):
                        trp = pt.tile([P, P], f32, space="PSUM", tag="tr")
                        nc.tensor.transpose(out=trp[:],
                                            in_=z[:, k * P:(k + 1) * P],
                                            identity=ident[:])
                        nc.vector.tensor_copy(out=zT[:, k * P:(k + 1) * P],
                                              in_=trp[:])
                    ps = pg.tile([P, H], f32, space="PSUM", tag="gemm")
                    for k in range(KH):
                        for nh in range(2):
                            nc.tensor.matmul(
                                out=ps[:, nh * 512:(nh + 1) * 512],
                                lhsT=zT[:, k * P:(k + 1) * P],
                                rhs=W_sb[:, k * H + nh * 512: k * H + (nh + 1) * 512],
                                start=(k == 0), stop=(k == KH - 1))
                    if l < L - 1:
                        ex_t = zp.tile([P, H], f16, tag="ex")
                        nc.scalar.activation(out=ex_t[:], in_=ps[:],
                                             func=mybir.ActivationFunctionType.Relu,
                                             scale=dinv_sb[:, t:t + 1])
                        nc.sync.dma_start(out=exch_in[t * P:(t + 1) * P, :],
                                          in_=ex_t[:])
                    else:
                        # h8 tile (f32r) -> logits -> log_softmax -> out
                        h8 = zp.tile([P, H], f32, tag="z")
                        nc.scalar.activation(out=h8[:], in_=ps[:],
                                             func=mybir.ActivationFunctionType.Relu)
                        hT = zp.tile([P, KH * P], f32r, tag="zT")
                        for k in range(KH):
                            trp = pt.tile([P, P], f32, space="PSUM", tag="tr")
                            nc.tensor.transpose(out=trp[:],
                                                in_=h8[:, k * P:(k + 1) * P],
                                                identity=ident[:])
                            nc.vector.tensor_copy(out=hT[:, k * P:(k + 1) * P],
                                                  in_=trp[:])
                        psl = pt.tile([P, C], f32, space="PSUM", tag="tr")
                        for k in range(KH):
                            nc.tensor.matmul(
                                out=psl[:],
                                lhsT=hT[:, k * P:(k + 1) * P],
                                rhs=W2_sb[:, k * C:(k + 1) * C],
                                start=(k == 0), stop=(k == KH - 1))
                        nc.vector.tensor_add(out=psl[:], in0=psl[:], in1=b2_sb[:])
                        mx = zp.tile([P, 1], f32, tag="mx")
                        nc.vector.tensor_reduce(out=mx[:], in_=psl[:],
                                                axis=mybir.AxisListType.X,
                                                op=mybir.AluOpType.max)
                        nmx = zp.tile([P, 1], f32, tag="nmx")
                        nc.vector.tensor_scalar(
                            out=nmx[:], in0=mx[:], scalar1=-1.0, scalar2=None,
                            op0=mybir.AluOpType.mult)
                        esb = zp.tile([P, C], f32, tag="esb")
                        se = zp.tile([P, 1], f32, tag="se")
                        nc.scalar.activation(out=esb[:], in_=psl[:],
                                             func=mybir.ActivationFunctionType.Exp,
                                             bias=nmx[:], accum_out=se[:])
                        lse = zp.tile([P, 1], f32, tag="lse")
                        nc.scalar.activation(out=lse[:], in_=se[:],
                                             func=mybir.ActivationFunctionType.Ln)
                        o_t = zp.tile([P, C], f32, tag="ot")
                        nc.vector.tensor_scalar(
                            out=o_t[:], in0=psl[:], scalar1=mx[:], scalar2=lse[:],
                            op0=mybir.AluOpType.subtract,
                            op1=mybir.AluOpType.subtract)
                        nc.sync.dma_start(out=t_out[t * P:(t + 1) * P, :],
                                          in_=o_t[:])
                if l < L - 1:
                    nc.gpsimd.collective_compute(
                        "AllGather", mybir.AluOpType.bypass,
                        replica_groups=[list(range(NCORES))],
                        ins=[exch_in.ap().opt()],
                        outs=[tables[(l + 1) % 2].ap().opt()])
    nc.compile()
    return nc


def kernel(**inputs):
    in_maps, nchunk = _preprocess(
        inputs["x"], inputs["edge_index"], inputs["W1"], inputs["b1"],
        inputs["Wg"], inputs["W2"], inputs["b2"])
    key = ("nc", nchunk)
    if key not in _cache:
        _cache[key] = _build(nchunk)
    nc = _cache[key]
    res = run_bass_kernel_spmd(nc, in_maps, list(range(NCORES)))
    out = np.concatenate(
        [res.results[c]["out"][:SH] for c in range(NCORES)], axis=0)
    return out.astype(np.float32)



# revision 8
# speedup vs baseline: 2.1906x; 2.1906x over previous
"""GCNII forward on 8 TRN2 NeuronCores (self-contained).

Strategy (1D row partitioning):
- nodes sharded 2500/core (padded 2560); edges assigned to the core owning dst.
- per layer: ht = dinv*h exchanged as fp8e4m3 via two half AllGathers into a
  shared DRAM table [20480,1024]; each core indirect-DMA-gathers its edges'
  source rows in batches of 6 chunks (768 rows) per instruction, scatter-adds
  them into per-dst-tile PSUM via one-hot fp8 DoubleRow matmuls (256 edges per
  accumulation step), then z = 0.9*dinv*agg + 0.1*h0 (h0 SBUF-resident, f16)
  and the layer GEMM z @ ((1-b)I + b*W) in f16 with the identity residual
  folded into the weights on the host. z transposed via f16 PE transposes.
- self-loops are real edges; gcn_norm folded into per-node dinv scaling.
"""
import math
import numpy as np

import concourse.bass as bass
import concourse.mybir as mybir
import concourse.tile as tile
from concourse import bacc
from concourse.bass_utils import run_bass_kernel_spmd
from concourse.masks import make_identity

# problem constants (hardcoded per contract)
N, E = 20000, 320000
F_IN, H, C, L = 512, 1024, 64, 8
ALPHA, THETA = 0.1, 0.5
NCORES = 8
SH = N // NCORES          # 2500 real rows per core
SHP = 2560                # padded rows per core (20*128)
HALF = SHP // 2           # 1280 rows per AllGather half
V = NCORES * SHP          # padded table rows
P = 128
NT = SHP // P             # 20 dst tiles per core
KF = F_IN // P            # 4 k-tiles for W1
KH = H // P               # 8 k-tiles for H
GK = 6                    # chunks gathered per indirect DMA

f32 = mybir.dt.float32
f16 = mybir.dt.float16
f8 = mybir.dt.float8e4
i16 = mybir.dt.int16

_cache = {}


def _f8(a):
    import ml_dtypes
    return np.asarray(a, np.float32).astype(ml_dtypes.float8_e4m3fn).view(
        np.uint8)


def _preprocess(x, edge_index, W1, b1, Wg, W2, b2):
    src = np.asarray(edge_index[0], dtype=np.int64)
    dst = np.asarray(edge_index[1], dtype=np.int64)
    loops = np.arange(N, dtype=np.int64)
    src = np.concatenate([src, loops])
    dst = np.concatenate([dst, loops])
    deg = np.bincount(dst, minlength=N).astype(np.float32)
    dinv = 1.0 / np.sqrt(np.maximum(deg, 1.0))

    core = dst // SH
    d_loc = dst - core * SH
    tl = d_loc // P
    slot = d_loc % P
    gid = core * NT + tl
    order = np.argsort(gid, kind="stable")
    gid_s = gid[order]
    src_s = src[order]
    slot_s = slot[order]
    counts = np.bincount(gid_s, minlength=NCORES * NT)
    starts = np.concatenate([[0], np.cumsum(counts)[:-1]])
    j = np.arange(len(gid_s)) - starts[gid_s]
    nchunk = int(math.ceil(counts.max() / P))
    nchunk = GK * int(math.ceil(nchunk / GK))  # pad to gather batch multiple
    c_idx = j // P
    p_idx = j % P
    # table row for global node n owned by core c at local i:
    # half = i // HALF; row = half*(8*HALF) + c*HALF + (i - half*HALF)
    s_core = src_s // SH
    s_loc = src_s - s_core * SH
    s_half = s_loc // HALF
    s_tab = (s_half * (NCORES * HALF) + s_core * HALF
             + (s_loc - s_half * HALF)).astype(np.int32)

    offs = np.zeros((NCORES, P, NT * nchunk), dtype=np.int32)
    S = np.zeros((NCORES, NT, P, nchunk, P), dtype=np.float32)
    core_s = gid_s // NT
    tl_s = gid_s % NT
    offs[core_s, p_idx, tl_s * nchunk + c_idx] = s_tab
    S[core_s, tl_s, p_idx, c_idx, slot_s] = 1.0
    S = S.reshape(NCORES, NT, P, nchunk * P)
    # dma_gather idxs: per tile, flat order i = chunk*128 + slot, wrapped
    # into 16 partitions ([i%16, i//16]) and replicated to 128.
    ncols = nchunk * P // 16
    idxs = np.zeros((NCORES, 16, NT * ncols), dtype=np.int16)
    ii = np.arange(nchunk * P)
    for c in range(NCORES):
        flat_all = offs[c].reshape(P, NT, nchunk).transpose(1, 2, 0)
        for t in range(NT):
            f = flat_all[t].reshape(-1)
            w = np.zeros((16, ncols), np.int16)
            w[ii % 16, ii // 16] = f.astype(np.int16)
            idxs[c, :, t * ncols:(t + 1) * ncols] = w
    idxs = np.tile(idxs, (1, 8, 1))

    dinv_pad = np.zeros(NCORES * SHP, dtype=np.float32)
    idx = np.arange(N)
    dinv_pad[(idx // SH) * SHP + (idx % SH)] = dinv
    dinvc = dinv_pad.reshape(NCORES, NT, P).transpose(0, 2, 1).copy()
    dinv09c = (0.9 * dinvc).astype(np.float32)

    x = np.asarray(x, dtype=np.float32)
    xT = np.zeros((NCORES, F_IN, SHP), dtype=np.float16)
    for c in range(NCORES):
        xT[c, :, :SH] = x[c * SH:(c + 1) * SH].T.astype(np.float16)

    betas = np.log(THETA / np.arange(1.0, L + 1.0, dtype=np.float64) + 1.0)
    Wg = np.asarray(Wg, dtype=np.float64)
    eye = np.eye(H, dtype=np.float64)
    Wt = np.stack([(1.0 - betas[l]) * eye + betas[l] * Wg[l] for l in range(L)])
    Wt = Wt.astype(np.float16)

    b1b = np.broadcast_to(np.asarray(b1, np.float32), (P, H)).copy()
    b2b = np.broadcast_to(np.asarray(b2, np.float32), (P, C)).copy()

    in_maps = []
    for c in range(NCORES):
        in_maps.append({
            "xT": xT[c],
            "W1": np.asarray(W1, np.float16),
            "Wt": Wt,
            "W2": np.asarray(W2, np.float16),
            "b1b": b1b,
            "b2b": b2b,
            "dinvc": dinvc[c],
            "dinv09c": dinv09c[c],
            "idxs": idxs[c],
            "Smat": _f8(S[c]),
        })
    return in_maps, nchunk


def _build(nchunk):
    npair = nchunk // 2
    nc = bacc.Bacc("TRN2", target_bir_lowering=False, debug=False,
                   num_devices=NCORES)
    t_xT = nc.dram_tensor("xT", [F_IN, SHP], f16, kind="ExternalInput")
    t_W1 = nc.dram_tensor("W1", [F_IN, H], f16, kind="ExternalInput")
    t_Wt = nc.dram_tensor("Wt", [L, H, H], f16, kind="ExternalInput")
    t_W2 = nc.dram_tensor("W2", [H, C], f16, kind="ExternalInput")
    t_b1 = nc.dram_tensor("b1b", [P, H], f32, kind="ExternalInput")
    t_b2 = nc.dram_tensor("b2b", [P, C], f32, kind="ExternalInput")
    t_dinv = nc.dram_tensor("dinvc", [P, NT], f32, kind="ExternalInput")
    t_dinv09 = nc.dram_tensor("dinv09c", [P, NT], f32, kind="ExternalInput")
    t_idx = nc.dram_tensor("idxs", [P, NT * nchunk * P // 16], i16,
                           kind="ExternalInput")
    t_S = nc.dram_tensor("Smat", [NT, P, nchunk * P], f8, kind="ExternalInput")
    t_out = nc.dram_tensor("out", [SHP, C], f32, kind="ExternalOutput")

    exch_in = nc.dram_tensor("exch", [SHP, H], f8)
    tables = [nc.dram_tensor(f"tbl{i}", [V, H], f8, addr_space="Shared")
              for i in range(2)]

    with tile.TileContext(nc) as tc:
        with (
            tc.tile_pool(name="const", bufs=1) as cp,
            tc.tile_pool(name="wpool", bufs=2) as wp,
            tc.tile_pool(name="gpool", bufs=3) as gp,
            tc.tile_pool(name="spool", bufs=3) as sp,
            tc.tile_pool(name="zpool", bufs=2) as zp,
            tc.tile_pool(name="ps_agg", bufs=2, space="PSUM") as pa,
            tc.tile_pool(name="ps_gemm", bufs=1, space="PSUM") as pg,
            tc.tile_pool(name="ps_tr", bufs=2, space="PSUM") as pt,
        ):
            ident = cp.tile([P, P], f16, tag="ident")
            make_identity(nc, ident[:])
            idx_sb = cp.tile([P, NT * nchunk * P // 16], i16, tag="idx")
            nc.sync.dma_start(out=idx_sb[:], in_=t_idx[:])
            dinv_sb = cp.tile([P, NT], f32, tag="dinv")
            nc.sync.dma_start(out=dinv_sb[:], in_=t_dinv[:])
            dinv09_sb = cp.tile([P, NT], f32, tag="dinv09")
            nc.sync.dma_start(out=dinv09_sb[:], in_=t_dinv09[:])
            b1_sb = cp.tile([P, H], f32, tag="b1")
            nc.sync.dma_start(out=b1_sb[:], in_=t_b1[:])
            b2_sb = cp.tile([P, C], f32, tag="b2")
            nc.sync.dma_start(out=b2_sb[:], in_=t_b2[:])
            W2_sb = cp.tile([P, KH * C], f16, tag="W2")
            for k in range(KH):
                nc.scalar.dma_start(out=W2_sb[:, k * C:(k + 1) * C],
                                    in_=t_W2[k * P:(k + 1) * P, :])
            # h0 resident (f16, 0.1-scaled): [P, NT*H]
            h0_sb = cp.tile([P, NT * H], f16, tag="h0")

            # ---- phase 0: h0 = relu(x@W1 + b1); table0 = f8(dinv*h0)
            xT_sb = cp.tile([P, KF * SHP], f16, tag="xT")
            for k in range(KF):
                nc.sync.dma_start(out=xT_sb[:, k * SHP:(k + 1) * SHP],
                                  in_=t_xT[k * P:(k + 1) * P, :])
            W1_sb = cp.tile([P, KF * H], f16, tag="W1")
            for k in range(KF):
                nc.scalar.dma_start(out=W1_sb[:, k * H:(k + 1) * H],
                                    in_=t_W1[k * P:(k + 1) * P, :])
            for t in range(NT):
                ps = pg.tile([P, H], f32, space="PSUM", tag="gemm")
                for k in range(KF):
                    for nh in range(2):
                        nc.tensor.matmul(
                            out=ps[:, nh * 512:(nh + 1) * 512],
                            lhsT=xT_sb[:, k * SHP + t * P: k * SHP + (t + 1) * P],
                            rhs=W1_sb[:, k * H + nh * 512: k * H + (nh + 1) * 512],
                            start=(k == 0), stop=(k == KF - 1))
                nc.vector.tensor_add(out=ps[:], in0=ps[:], in1=b1_sb[:])
                nc.scalar.activation(out=h0_sb[:, t * H:(t + 1) * H], in_=ps[:],
                                     func=mybir.ActivationFunctionType.Relu,
                                     scale=0.1)
                ex_t = zp.tile([P, H], f8, tag="ex")
                nc.scalar.activation(out=ex_t[:], in_=ps[:],
                                     func=mybir.ActivationFunctionType.Relu,
                                     scale=dinv_sb[:, t:t + 1])
                nc.sync.dma_start(out=exch_in[t * P:(t + 1) * P, :], in_=ex_t[:])
                if t == NT // 2 - 1:
                    nc.gpsimd.collective_compute(
                        "AllGather", mybir.AluOpType.bypass,
                        replica_groups=[list(range(NCORES))],
                        ins=[exch_in[0:HALF, :]],
                        outs=[tables[0][0:NCORES * HALF, :]])
            nc.gpsimd.collective_compute(
                "AllGather", mybir.AluOpType.bypass,
                replica_groups=[list(range(NCORES))],
                ins=[exch_in[HALF:SHP, :]],
                outs=[tables[0][NCORES * HALF:V, :]])

            # ---- layers
            for l in range(L):
                tbl = tables[l % 2]
                W_sb = wp.tile([P, KH * H], f16, tag="W")
                for k in range(KH):
                    nc.scalar.dma_start(out=W_sb[:, k * H:(k + 1) * H],
                                        in_=t_Wt[l, k * P:(k + 1) * P, :])
                for t in range(NT):
                    agg = pa.tile([P, H], f32, space="PSUM", tag="agg")
                    S_sb = sp.tile([P, nchunk * P], f8, tag="S")
                    nc.scalar.dma_start(out=S_sb[:], in_=t_S[t])
                    S3 = S_sb[:].rearrange("p (c d) -> p c d", c=nchunk)
                    g_sb = gp.tile([P, nchunk * H], f8, tag="g")
                    g3 = g_sb[:].rearrange("p (c h) -> p c h", c=nchunk)
                    tc0 = t * (nchunk * P // 16)
                    for gj in range(nchunk // GK):
                        nc.gpsimd.dma_gather(
                            g3[:, gj * GK:(gj + 1) * GK, :], tbl.ap(),
                            idx_sb[:, tc0 + gj * (GK * P // 16):
                                   tc0 + (gj + 1) * (GK * P // 16)],
                            GK * P, GK * P, H)
                    for pr in range(npair):
                        for nh in range(2):
                            nc.tensor.matmul(
                                out=agg[:, nh * 512:(nh + 1) * 512],
                                lhsT=S3[:, 2 * pr:2 * pr + 2, :],
                                rhs=g3[:, 2 * pr:2 * pr + 2,
                                       nh * 512:(nh + 1) * 512],
                                perf_mode=mybir.MatmulPerfMode.DoubleRow,
                                start=(pr == 0), stop=(pr == npair - 1))
                    # z = 0.9*dinv*agg + 0.1*h0   (f16)
                    z0 = zp.tile([P, H], f16, tag="z0")
                    nc.vector.tensor_scalar(
                        out=z0[:], in0=agg[:], scalar1=dinv09_sb[:, t:t + 1],
                        scalar2=None, op0=mybir.AluOpType.mult)
                    z = zp.tile([P, H], f16, tag="z")
                    nc.vector.tensor_add(out=z[:], in0=z0[:],
                                         in1=h0_sb[:, t * H:(t + 1) * H])
                    # transpose z -> zT (8 f16 PE transposes, one copy)
                    zT = zp.tile([P, KH * P], f16, tag="zT")
                    trp = pt.tile([P, KH * P], f16, space="PSUM", tag="tr")
                    for k in range(KH):
                        nc.tensor.transpose(out=trp[:, k * P:(k + 1) * P],
                                            in_=z[:, k * P:(k + 1) * P],
                                            identity=ident[:])
                    nc.vector.tensor_copy(out=zT[:], in_=trp[:])
                    ps = pg.tile([P, H], f32, space="PSUM", tag="gemm")
                    for k in range(KH):
                        for nh in range(2):
                            nc.tensor.matmul(
                                out=ps[:, nh * 512:(nh + 1) * 512],
                                lhsT=zT[:, k * P:(k + 1) * P],
                                rhs=W_sb[:, k * H + nh * 512: k * H + (nh + 1) * 512],
                                start=(k == 0), stop=(k == KH - 1))
                    if l < L - 1:
                        ex_t = zp.tile([P, H], f8, tag="ex")
                        nc.scalar.activation(out=ex_t[:], in_=ps[:],
                                             func=mybir.ActivationFunctionType.Relu,
                                             scale=dinv_sb[:, t:t + 1])
                        nc.sync.dma_start(out=exch_in[t * P:(t + 1) * P, :],
                                          in_=ex_t[:])
                        if t == NT // 2 - 1:
                            nc.gpsimd.collective_compute(
                                "AllGather", mybir.AluOpType.bypass,
                                replica_groups=[list(range(NCORES))],
                                ins=[exch_in[0:HALF, :]],
                                outs=[tables[(l + 1) % 2][0:NCORES * HALF, :]])
                        elif t == NT - 1:
                            nc.gpsimd.collective_compute(
                                "AllGather", mybir.AluOpType.bypass,
                                replica_groups=[list(range(NCORES))],
                                ins=[exch_in[HALF:SHP, :]],
                                outs=[tables[(l + 1) % 2][NCORES * HALF:V, :]])
                    else:
                        # h8 tile (f16) -> logits -> log_softmax -> out
                        h8 = zp.tile([P, H], f16, tag="z")
                        nc.scalar.activation(out=h8[:], in_=ps[:],
                                             func=mybir.ActivationFunctionType.Relu)
                        hT = zp.tile([P, KH * P], f16, tag="zT")
                        trp = pt.tile([P, KH * P], f16, space="PSUM", tag="tr")
                        for k in range(KH):
                            nc.tensor.transpose(out=trp[:, k * P:(k + 1) * P],
                                                in_=h8[:, k * P:(k + 1) * P],
                                                identity=ident[:])
                        nc.vector.tensor_copy(out=hT[:], in_=trp[:])
                        psf = pg.tile([P, H], f32, space="PSUM", tag="gemm")
                        psl = psf[:, 0:C]
                        for k in range(KH):
                            nc.tensor.matmul(
                                out=psl,
                                lhsT=hT[:, k * P:(k + 1) * P],
                                rhs=W2_sb[:, k * C:(k + 1) * C],
                                start=(k == 0), stop=(k == KH - 1))
                        nc.vector.tensor_add(out=psl, in0=psl, in1=b2_sb[:])
                        mx = zp.tile([P, 1], f32, tag="mx")
                        nc.vector.tensor_reduce(out=mx[:], in_=psl,
                                                axis=mybir.AxisListType.X,
                                                op=mybir.AluOpType.max)
                        nmx = zp.tile([P, 1], f32, tag="nmx")
                        nc.vector.tensor_scalar(
                            out=nmx[:], in0=mx[:], scalar1=-1.0, scalar2=None,
                            op0=mybir.AluOpType.mult)
                        esb = zp.tile([P, C], f32, tag="esb")
                        se = zp.tile([P, 1], f32, tag="se")
                        nc.scalar.activation(out=esb[:], in_=psl,
                                             func=mybir.ActivationFunctionType.Exp,
                                             bias=nmx[:], accum_out=se[:])
                        lse = zp.tile([P, 1], f32, tag="lse")
                        nc.scalar.activation(out=lse[:], in_=se[:],
                                             func=mybir.ActivationFunctionType.Ln)
                        o_t = zp.tile([P, C], f32, tag="ot")
                        nc.vector.tensor_scalar(
                            out=o_t[:], in0=psl, scalar1=mx[:], scalar2=lse[:],
                            op0=mybir.AluOpType.subtract,
                            op1=mybir.AluOpType.subtract)
                        nc.sync.dma_start(out=t_out[t * P:(t + 1) * P, :],
                                          in_=o_t[:])
    nc.compile()
    return nc


def kernel(**inputs):
    in_maps, nchunk = _preprocess(
        inputs["x"], inputs["edge_index"], inputs["W1"], inputs["b1"],
        inputs["Wg"], inputs["W2"], inputs["b2"])
    key = ("nc", nchunk)
    if key not in _cache:
        _cache[key] = _build(nchunk)
    nc = _cache[key]
    res = run_bass_kernel_spmd(nc, in_maps, list(range(NCORES)))
    out = np.concatenate(
        [res.results[c]["out"][:SH] for c in range(NCORES)], axis=0)
    return out.astype(np.float32)
